# revision 1
# baseline (speedup 1.0000x reference)
"""Multi-head attention (B=2, S=2048, D=1024, H=16, Hd=64) on 8 Trainium2
NeuronCores.

Sharding: 8 cores = (batch 2) x (head-half 2) x (q-half 2).
Core (b, hh, qh) computes, for batch b, heads hh*8..hh*8+8 and query rows
qh*1024..qh*1024+1024, the partial output

    outp = (softmax-attention of its heads restricted to its q rows) @ Wo_part.T
           + bo_part

and the host sums the two head-half partials per (b, qh) block.  bo is fed as
zeros to the hh==1 cores so the bias is counted once.

Device-side layouts (host pre-transposes so every matmul is a natural
lhsT.T @ rhs with the contraction dim on SBUF partitions):
  xT    [D, S]      x[b].T
  wqT/wkT/wvT [D, 512]  W.T column slice for this head-half
  woT   [512, D]    Wo.T row slice for this head-half
  maskT [S, 1024]   mask[b,0].T column slice for this q-half (int32)

Pipeline per core:
  1. qT = (wqT.T @ xT-cols)  [512, 1024],  kT [512, 2048], V [2048, 512]
     all fp32r (full PE speed, ~1e-4 matmul error).
  2. Per head h, per s_k tile i: scoresT tile [128, 1024] = kT_h_i.T @ qT_h
     (K=64; head pairs land on PE row-groups 0-63/64-127 and run
     concurrently), exp on ScalarE (scale=1/8) -> bf16, mask multiply on
     VectorE (bf16, 2x mode), then attnV accumulation
     out_ps [128, 512] += V_aug_i.T @ expm  where V_aug has 64 ones
     columns so PSUM rows 64..127 all hold Z = sum(expm); reciprocal of
     those rows gives 1/Z already replicated across partitions.
  3. out partial [1024, 1024] = out_cT.T @ woT (+ bo broadcast), DMA out.

No collectives; the only cross-core step is the host-side partial sum.
"""

import sys

if "/opt/trn_rl_repo" not in sys.path:
    sys.path.insert(0, "/opt/trn_rl_repo")

import numpy as np

B, S, D = 2, 2048, 1024
H, HD = 16, 64
NCORES = 8
HPC = 8  # heads per core
DPC = HPC * HD  # 512 head dims per core
SQC = S // 2  # 1024 q rows per core
KT = D // 128  # 8 contraction tiles
NSK = S // 128  # 16 s_k tiles
NDB = DPC // 128  # 4 d-blocks of the per-core head dims

_CACHE = {}


def _build():
    import concourse.bacc as bacc
    import concourse.mybir as mybir
    import concourse.tile as tile

    F32 = mybir.dt.float32
    F32R = mybir.dt.float32r
    BF16 = mybir.dt.bfloat16
    I32 = mybir.dt.int32
    MULT = mybir.AluOpType.mult
    ADD = mybir.AluOpType.add
    EQ = mybir.AluOpType.is_equal
    EXP = mybir.ActivationFunctionType.Exp

    nc = bacc.Bacc("TRN2", target_bir_lowering=False, debug=False)

    xT = nc.dram_tensor("xT", [D, S], F32, kind="ExternalInput")
    wqT = nc.dram_tensor("wqT", [D, DPC], F32, kind="ExternalInput")
    wkT = nc.dram_tensor("wkT", [D, DPC], F32, kind="ExternalInput")
    wvT = nc.dram_tensor("wvT", [D, DPC], F32, kind="ExternalInput")
    woT = nc.dram_tensor("woT", [DPC, D], F32, kind="ExternalInput")
    maskT = nc.dram_tensor("maskT", [S, SQC], I32, kind="ExternalInput")
    bo = nc.dram_tensor("bo", [D], F32, kind="ExternalInput")
    outp = nc.dram_tensor("outp", [SQC, D], F32, kind="ExternalOutput")

    xT_r = xT.rearrange("(t p) s -> p t s", p=128)  # [128, KT, S]
    wqT_r = wqT.rearrange("(t p) d -> p t d", p=128)
    wkT_r = wkT.rearrange("(t p) d -> p t d", p=128)
    wvT_r = wvT.rearrange("(t p) d -> p t d", p=128)
    woT_r = woT.rearrange("(c p) d -> p c d", p=128)  # [128, NDB, D]
    maskT_r = maskT.rearrange("(i p) q -> p i q", p=128)  # [128, NSK, SQC]

    NM_KEEP = NSK  # all mask tiles fit in the keep pool now

    with tile.TileContext(nc) as tc:
        with tc.tile_pool(name="keep", bufs=1) as keep:
            # ---- persistent SBUF tensors --------------------------------
            qT_sb = keep.tile([128, NDB, SQC], F32R)  # 16KB/part
            kT_sb = keep.tile([128, NDB, S], F32R)  # 32KB/part
            v_aug = keep.tile([128, NSK, HPC * 128], BF16)  # 32KB/part
            out_cT = keep.tile([128, NDB, SQC], F32R)  # 16KB/part

            # ones block of V_aug (overwritten below on the V columns)
            nc.vector.memset(v_aug[:], 1.0)

            # mask conversion pipeline: int32 0/1 -> bf16 (mask==0 -> 1.0).
            # Separate tile per s_k block so consumers start as soon as
            # their block is converted; the first NM_KEEP live in this pool
            # (addresses disjoint from phase 1) so they convert early.
            mask01 = [None] * NSK

            def emit_mask(pool, mpool, i):
                # int32 mask halves ride the HWDGE queues (so they stay
                # behind the x loads emitted first); DVE is_equal converts
                # to the bf16 keep-mask.
                m = pool.tile([128, SQC], BF16, tag=f"m{i}", name=f"mask01_{i}")
                for half in range(2):
                    sl = slice(half * (SQC // 2), (half + 1) * (SQC // 2))
                    mi = mpool.tile([128, SQC // 2], I32, tag="mi")
                    nc.sync.dma_start(out=mi[:], in_=maskT_r[:, i, sl])
                    nc.vector.tensor_scalar(
                        out=m[:, sl],
                        in0=mi[:],
                        scalar1=0,
                        scalar2=None,
                        op0=EQ,
                    )
                mask01[i] = m

            # ---- phase 1: projections (all fp32r) -----------------------
            # Split over k into two rounds of 4 k-tiles so each PSUM
            # accumulation group only spans half the x stream: round A
            # (k-tiles 0-3) evicts partial sums with a copy, round B
            # (k-tiles 4-7) finishes with an add.  x_sb has 6 slots so
            # round B's first two chunks prefetch during round A.
            with (
                tc.tile_pool(name="p1", bufs=1) as p1,
                tc.tile_pool(name="wslot", bufs=1) as wslot,
                tc.tile_pool(name="stg", bufs=2) as stg,
                tc.tile_pool(name="mstage", bufs=2) as mstage,
                tc.tile_pool(name="ps1", bufs=6, space="PSUM") as ps1,
            ):
                XS = 5
                x_sb = p1.tile([128, XS, S], F32R)  # 48KB/part
                _flip = [0]

                def stage_convert(dram_ap, dst_ap):
                    st = stg.tile([128, 1024], F32, tag="xs")
                    src = st[:]
                    if len(dst_ap.shape) == 3:
                        src = src.rearrange(
                            "p (a b) -> p a b", b=dst_ap.shape[2]
                        )
                    nc.sync.dma_start(out=st[:], in_=dram_ap)
                    _flip[0] ^= 1
                    if _flip[0]:
                        nc.vector.tensor_copy(dst_ap, src)
                    else:
                        nc.scalar.copy(dst_ap, src)

                def load_w_half(src_r, nm, rnd):
                    # gpsimd cast-DMA: fp32 HBM -> fp32r SBUF directly.
                    w = wslot.tile(
                        [128, KT // 2, DPC], F32R, tag=f"w{nm}", name=f"w_{nm}{rnd}"
                    )
                    nc.gpsimd.dma_start(
                        out=w[:], in_=src_r[:, rnd * 4 : (rnd + 1) * 4, :]
                    )
                    return w

                for rnd in range(2):
                    wq = load_w_half(wqT_r, "q", rnd)
                    wk = load_w_half(wkT_r, "k", rnd)
                    wv = load_w_half(wvT_r, "v", rnd)
                    for tt in range(4):
                        t = rnd * 4 + tt
                        for c in range(2):
                            stage_convert(
                                xT_r[:, t, c * 1024 : (c + 1) * 1024],
                                x_sb[:, t % XS, c * 1024 : (c + 1) * 1024],
                            )
                    if rnd == 1:
                        for i in range(NSK):
                            emit_mask(keep, mstage, i)

                    def evict(dst_ap, ps_ap):
                        if rnd == 0:
                            nc.any.tensor_copy(dst_ap, ps_ap)
                        else:
                            nc.vector.tensor_tensor(
                                out=dst_ap, in0=ps_ap, in1=dst_ap, op=ADD
                            )

                    def emit_q(db):
                        for jq in range(SQC // 512):
                            ps = ps1.tile([128, 512], F32, tag="ps")
                            for tt in range(4):
                                t = rnd * 4 + tt
                                nc.tensor.matmul(
                                    ps[:],
                                    wq[:, tt, db * 128 : (db + 1) * 128],
                                    x_sb[:, t % XS, jq * 512 : (jq + 1) * 512],
                                    start=(tt == 0),
                                    stop=(tt == 3),
                                )
                            evict(qT_sb[:, db, jq * 512 : (jq + 1) * 512], ps[:])

                    def emit_k(db):
                        for jk in range(S // 512):
                            ps = ps1.tile([128, 512], F32, tag="ps")
                            for tt in range(4):
                                t = rnd * 4 + tt
                                nc.tensor.matmul(
                                    ps[:],
                                    wk[:, tt, db * 128 : (db + 1) * 128],
                                    x_sb[:, t % XS, jk * 512 : (jk + 1) * 512],
                                    start=(tt == 0),
                                    stop=(tt == 3),
                                )
                            evict(kT_sb[:, db, jk * 512 : (jk + 1) * 512], ps[:])

                    def emit_v(sb):
                        ps = ps1.tile([128, 512], F32, tag="ps")
                        for tt in range(4):
                            t = rnd * 4 + tt
                            nc.tensor.matmul(
                                ps[:],
                                x_sb[:, t % XS, sb * 128 : (sb + 1) * 128],
                                wv[:, tt, :],
                                start=(tt == 0),
                                stop=(tt == 3),
                            )
                        evict(
                            v_aug[:, sb, :]
                            .rearrange("p (h c) -> p h c", h=HPC)[:, :, 0:HD],
                            ps[:].rearrange("p (h c) -> p h c", h=HPC),
                        )

                    if rnd == 0:
                        for db in range(NDB):
                            emit_q(db)
                        for db in range(NDB):
                            emit_k(db)
                        for sb in range(NSK):
                            emit_v(sb)
                    else:
                        # round B ordered so phase 2 (which needs v_aug and
                        # the low head-pair blocks first) can start early.
                        for sb in range(NSK):
                            emit_v(sb)
                        for db in range(NDB):
                            emit_k(db)
                            emit_q(db)

            # ---- phases 2+3 (interleaved) -------------------------------
            with (
                tc.tile_pool(name="p2", bufs=3) as p2,
                tc.tile_pool(name="pexpm", bufs=4) as pexpm,
                tc.tile_pool(name="p3", bufs=1) as p3,
                tc.tile_pool(name="p3w", bufs=3) as p3w,
                tc.tile_pool(name="sc", bufs=2, space="PSUM") as scp,
                tc.tile_pool(name="op", bufs=2, space="PSUM") as opp,
                tc.tile_pool(name="ps3", bufs=2, space="PSUM") as ps3,
            ):
                wo_sb = p3.tile([128, NDB, D], F32R)
                nc.gpsimd.dma_start(out=wo_sb[:], in_=woT_r[:])
                bo_rep = p3.tile([128, D], F32)
                nc.sync.dma_start(
                    out=bo_rep[:], in_=bo.ap()[None, :].to_broadcast((128, D))
                )

                def emit_phase3(ms):
                    # output projection for s_q blocks `ms` (their out_cT
                    # columns are complete); interleaves with phase 2.
                    for m in ms:
                        for n in range(D // 512):
                            ps = ps3.tile([128, 512], F32, tag="ps3")
                            for c in range(NDB):
                                nc.tensor.matmul(
                                    ps[:],
                                    out_cT[:, c, m * 128 : (m + 1) * 128],
                                    wo_sb[:, c, n * 512 : (n + 1) * 512],
                                    start=(c == 0),
                                    stop=(c == NDB - 1),
                                )
                            ob = p3w.tile([128, 512], F32, tag="ob")
                            nc.vector.tensor_tensor(
                                out=ob[:],
                                in0=ps[:],
                                in1=bo_rep[:, n * 512 : (n + 1) * 512],
                                op=ADD,
                            )
                            nc.sync.dma_start(
                                out=outp[
                                    m * 128 : (m + 1) * 128,
                                    n * 512 : (n + 1) * 512,
                                ],
                                in_=ob[:],
                            )

                # software pipeline over i; j outer so each j-half of
                # out_cT completes early and its output projection runs
                # under the other half's attention.
                LOOKAHEAD = 1
                for j in range(2):  # s_q half
                    jsl = slice(j * 512, (j + 1) * 512)
                    for hp in range(HPC // 2):  # head pairs
                        out_ps = [
                            opp.tile([128, 512], F32, tag="ops", name=f"ops_{hp}_{j}_{h2}")
                            for h2 in range(2)
                        ]
                        expm_q = {}
                        for ii in range(NSK + LOOKAHEAD):
                            if ii < NSK:
                                i = ii
                                sc = scp.tile(
                                    [128, 2, 512], F32, tag="sc", name=f"sc_{hp}_{j}_{i}"
                                )
                                for h2 in range(2):
                                    nc.tensor.matmul(
                                        sc[:, h2, :],
                                        kT_sb[
                                            h2 * 64 : (h2 + 1) * 64,
                                            hp,
                                            i * 128 : (i + 1) * 128,
                                        ],
                                        qT_sb[h2 * 64 : (h2 + 1) * 64, hp, jsl],
                                        start=True,
                                        stop=True,
                                    )
                                expt = p2.tile([128, 2, 512], BF16, tag="expt")
                                nc.scalar.activation(
                                    out=expt[:], in_=sc[:], func=EXP, scale=0.125
                                )
                                expm = pexpm.tile(
                                    [128, 2, 512],
                                    BF16,
                                    tag="expm",
                                    name=f"expm_{hp}_{j}_{i}",
                                )
                                for h2 in range(2):
                                    nc.vector.tensor_tensor(
                                        out=expm[:, h2, :],
                                        in0=expt[:, h2, :],
                                        in1=mask01[i][:, jsl],
                                        op=MULT,
                                    )
                                expm_q[i] = expm
                            if ii >= LOOKAHEAD:
                                i = ii - LOOKAHEAD
                                expm = expm_q.pop(i)
                                for h2 in range(2):
                                    h = 2 * hp + h2
                                    nc.tensor.matmul(
                                        out_ps[h2][:],
                                        v_aug[:, i, h * 128 : (h + 1) * 128],
                                        expm[:, h2, :],
                                        start=(i == 0),
                                        stop=(i == NSK - 1),
                                    )
                        # normalize: rows 64..127 of out_ps hold Z replicated;
                        # cheap approx reciprocal of one row, broadcast on
                        # gpsimd, multiply into out_cT.
                        for h2 in range(2):
                            zrow = p2.tile([1, 512], F32, tag="zrow")
                            nc.vector.tensor_copy(zrow[:], out_ps[h2][64:65, :])
                            zr1 = p2.tile([1, 512], F32, tag="zr1")
                            nc.vector.reciprocal_approx_fast(
                                out=zr1[:], in_=zrow[:]
                            )
                            zr = p2.tile([64, 512], F32, tag="zr")
                            nc.gpsimd.partition_broadcast(zr[:], zr1[:])
                            nc.vector.tensor_tensor(
                                out=out_cT[h2 * 64 : (h2 + 1) * 64, hp, jsl],
                                in0=out_ps[h2][0:64, :],
                                in1=zr[:],
                                op=MULT,
                            )
                    emit_phase3(range(j * 4, (j + 1) * 4))


    nc.compile()
    return nc


def _get_nc():
    if "nc" not in _CACHE:
        _CACHE["nc"] = _build()
    return _CACHE["nc"]


def _prep_inputs(x, mask, Wq, Wk, Wv, Wo, bo):
    """Build the 8 per-core input maps."""
    x = np.asarray(x, dtype=np.float32)
    mask = np.asarray(mask, dtype=np.int32)
    bo = np.asarray(bo, dtype=np.float32)
    wqT = np.ascontiguousarray(np.asarray(Wq, np.float32).T)
    wkT = np.ascontiguousarray(np.asarray(Wk, np.float32).T)
    wvT = np.ascontiguousarray(np.asarray(Wv, np.float32).T)
    woT = np.ascontiguousarray(np.asarray(Wo, np.float32).T)
    bz = np.zeros_like(bo)

    # The SPMD program always reads q activations from xT columns 0..SQC,
    # so qh==1 cores get xT rolled by -SQC along s (and maskT rows rolled
    # identically).  Attention sums over s_k, so a consistent permutation
    # of the k/V order (with the mask following it) leaves the result
    # unchanged.
    xTs = [np.ascontiguousarray(x[b].T) for b in range(B)]
    xTs_r = [np.ascontiguousarray(np.roll(t, -SQC, axis=1)) for t in xTs]
    maskTs = [np.ascontiguousarray(mask[b, 0].T) for b in range(B)]
    maskTs_r = [np.roll(t, -SQC, axis=0) for t in maskTs]

    in_maps = []
    for c in range(NCORES):
        b, hh, qh = c >> 2, (c >> 1) & 1, c & 1
        doff = hh * DPC
        qoff = qh * SQC
        mT = maskTs[b] if qh == 0 else maskTs_r[b]
        in_maps.append(
            {
                "xT": xTs[b] if qh == 0 else xTs_r[b],
                "wqT": np.ascontiguousarray(wqT[:, doff : doff + DPC]),
                "wkT": np.ascontiguousarray(wkT[:, doff : doff + DPC]),
                "wvT": np.ascontiguousarray(wvT[:, doff : doff + DPC]),
                "woT": np.ascontiguousarray(woT[doff : doff + DPC, :]),
                "maskT": np.ascontiguousarray(mT[:, qoff : qoff + SQC]),
                "bo": bo if hh == 0 else bz,
            }
        )
    return in_maps


def run(inputs: dict, trace: bool = False):
    """Run the kernel; returns (full_output, BassKernelResults)."""
    from concourse.bass_utils import run_bass_kernel_spmd

    nc = _get_nc()
    in_maps = _prep_inputs(**inputs)
    res = run_bass_kernel_spmd(
        nc, in_maps, core_ids=list(range(NCORES)), trace=trace
    )
    out = np.empty((B, S, D), dtype=np.float32)
    for b in range(B):
        for qh in range(2):
            c0 = (b << 2) | (0 << 1) | qh
            c1 = (b << 2) | (1 << 1) | qh
            out[b, qh * SQC : (qh + 1) * SQC, :] = (
                res.results[c0]["outp"] + res.results[c1]["outp"]
            )
    return out, res


def kernel(**inputs) -> np.ndarray:
    out, _ = run(inputs, trace=False)
    return out



# revision 8
# speedup vs baseline: 1.2005x; 1.2005x over previous
"""Multi-head attention (B=2, S=2048, D=1024, H=16, Hd=64) on 8 Trainium2
NeuronCores.

Sharding: 8 cores = (batch 2) x (head-quarter 4).  Core (b, hq) computes,
for batch b and heads hq*4..hq*4+3, the full-sequence partial output

    outp = (softmax-attention of its 4 heads over all 2048 q rows) @ Wo_part.T

and the host sums the four head-quarter partials per batch and adds bo.
No K/V projection is duplicated (unlike a q-split layout), so phase-1
tensor work is exactly 1/8 of the global total per core.

Everything is bf16 on the wire and in SBUF (PSUM accumulates fp32):
  xT     [D, S]    x[b].T                          bf16, 4MB
  wqT/wkT/wvT [D, 256]  W.T column slice           bf16
  woT    [256, D]  Wo.T row slice                  bf16
  maskT  [S, S]    keep-mask (mask[b,0]==0).T      bf16 0/1, 8MB
  outp   [S, D]    partial output                  fp32 (host sums + bo)

Pipeline per core (all heads packed as 2 head-pairs on partition halves
0-63 / 64-127 so the two scores matmuls of a pair run concurrently on
disjoint PE row groups):
  1. projections: qT/kT [128, 2, S], V [s, 256] in 8-k-tile PSUM chains;
     x is DMA'd in 4 column chunks [128, 8, 512] so chains (which contract
     over all 8 k-tiles but only 512 columns) start as soon as chunk 0
     lands.  V lands in v_aug [128 s, 16 sb, head*128 + (64 V | 64 ones)];
     the ones columns make attnV accumulate Z = sum(expm) into PSUM rows
     64..127 for free.
  2. per (head-pair c, q-chunk j of 512), 16 s_k tiles i:
     scT [128, 2, 512] = k_i.T @ q_j (two 64-row matmuls, concurrent),
     exp on ScalarE (scale=1/8, bf16 out), keep-mask multiply on VectorE,
     attnV accumulation out_ps [128, 2, 512] += V_aug_i.T @ expm.
     Normalize: reciprocal of Z rows 64..127, multiply into out_cT bf16.
  3. output projection: psum [128 q, 512 d] = out_cT.T @ woT, direct
     PSUM->DRAM DMA (bias bo is added on the host).

Phase-1 chains and phase-3 blocks are interleaved into the phase-2 unit
stream to keep the PE dense (HAM warm) while ScalarE (exp, the critical
~133us engine) streams at its own rate.
"""

import sys

if "/opt/trn_rl_repo" not in sys.path:
    sys.path.insert(0, "/opt/trn_rl_repo")

import numpy as np

B, S, D = 2, 2048, 1024
H, HD = 16, 64
NCORES = 8
HPC = 4  # heads per core
DPC = HPC * HD  # 256 head dims per core
KT = D // 128  # 8 contraction tiles
NSK = S // 128  # 16 s_k tiles
NJ = S // 512  # 4 q chunks
NC2 = HPC // 2  # 2 head pairs

_CACHE = {}


def _build():
    import concourse.bacc as bacc
    import concourse.mybir as mybir
    import concourse.tile as tile

    F32 = mybir.dt.float32
    BF16 = mybir.dt.bfloat16
    MULT = mybir.AluOpType.mult
    EXP = mybir.ActivationFunctionType.Exp

    nc = bacc.Bacc("TRN2", target_bir_lowering=False, debug=False)

    xT = nc.dram_tensor("xT", [D, S], BF16, kind="ExternalInput")
    wqT = nc.dram_tensor("wqT", [D, DPC], BF16, kind="ExternalInput")
    wkT = nc.dram_tensor("wkT", [D, DPC], BF16, kind="ExternalInput")
    wvT = nc.dram_tensor("wvT", [D, DPC], BF16, kind="ExternalInput")
    woT = nc.dram_tensor("woT", [DPC, D], BF16, kind="ExternalInput")
    maskT = nc.dram_tensor("maskT", [S, S], BF16, kind="ExternalInput")
    outp = nc.dram_tensor("outp", [S, D], BF16, kind="ExternalOutput")

    xT_r = xT.rearrange("(t p) s -> p t s", p=128)  # [128, KT, S]
    wqT_r = wqT.rearrange("(t p) d -> p t d", p=128)  # [128, KT, DPC]
    wkT_r = wkT.rearrange("(t p) d -> p t d", p=128)
    wvT_r = wvT.rearrange("(t p) d -> p t d", p=128)
    woT_r = woT.rearrange("(c p) d -> p c d", p=128)  # [128, 2, D]
    maskT_r = maskT.rearrange("(i p) q -> p i q", p=128)  # [128, NSK, S]

    with tile.TileContext(nc) as tc:
        with (
            tc.tile_pool(name="keep", bufs=1) as keep,
            tc.tile_pool(name="pexpt", bufs=3) as pexpt,
            tc.tile_pool(name="pexpm", bufs=4) as pexpm,
            tc.tile_pool(name="pnorm", bufs=2) as pnorm,
            tc.tile_pool(name="p3s", bufs=3) as p3s,
            tc.tile_pool(name="scp", bufs=2, space="PSUM") as scp,
            tc.tile_pool(name="opp", bufs=1, space="PSUM") as opp,
            tc.tile_pool(name="aux", bufs=2, space="PSUM") as aux,
        ):
            # ---- persistent SBUF ----------------------------------------
            x_sb = keep.tile([128, KT, S], BF16)  # 32KB/part
            wq_sb = keep.tile([128, KT, DPC], BF16)
            wk_sb = keep.tile([128, KT, DPC], BF16)
            wv_sb = keep.tile([128, KT, DPC], BF16)
            wo_sb = keep.tile([128, 2, D], BF16)
            qT_sb = keep.tile([128, NC2, S], BF16)
            kT_sb = keep.tile([128, NC2, S], BF16)
            v_aug = keep.tile([128, NSK, HPC * 128], BF16)  # 16KB/part
            mask01 = keep.tile([128, NSK, S], BF16)  # 64KB/part
            out_cT = keep.tile([128, NC2, S], BF16)

            # ---- DMAs (weights + x on SP HWDGE queue, mask on Pool SWDGE)
            nc.sync.dma_start(out=wq_sb[:], in_=wqT_r[:])
            nc.sync.dma_start(out=wk_sb[:], in_=wkT_r[:])
            nc.sync.dma_start(out=wv_sb[:], in_=wvT_r[:])
            nc.sync.dma_start(out=wo_sb[:], in_=woT_r[:])
            for jc in range(4):  # x column chunks [128, 8, 512], 1MB each
                sl = slice(jc * 512, (jc + 1) * 512)
                nc.sync.dma_start(out=x_sb[:, :, sl], in_=xT_r[:, :, sl])
            for i in range(NSK):  # mask s_k tiles [128, 2048], 0.5MB each
                nc.gpsimd.dma_start(out=mask01[:, i, :], in_=maskT_r[:, i, :])

            nc.any.memset(v_aug[:], 1.0)

            # ---- phase-1 chain emitters ---------------------------------
            def chain_kq(w_sb, dst_sb, c, jk):
                # dst[hd 0..127 of pair c, s jk*512..] over 8 k-tiles
                ps = aux.tile([128, 512], F32, tag="aux")
                sl = slice(jk * 512, (jk + 1) * 512)
                for t in range(KT):
                    nc.tensor.matmul(
                        ps[:],
                        w_sb[:, t, c * 128 : (c + 1) * 128],
                        x_sb[:, t, sl],
                        start=(t == 0),
                        stop=(t == KT - 1),
                    )
                nc.vector.tensor_copy(dst_sb[:, c, sl], ps[:])

            def chain_v(sb):
                ps = aux.tile([128, 256], F32, tag="aux")
                for t in range(KT):
                    nc.tensor.matmul(
                        ps[:],
                        x_sb[:, t, sb * 128 : (sb + 1) * 128],
                        wv_sb[:, t, :],
                        start=(t == 0),
                        stop=(t == KT - 1),
                    )
                nc.vector.tensor_copy(
                    v_aug[:, sb, :]
                    .rearrange("p (h c2) -> p h c2", h=HPC)[:, :, 0:HD],
                    ps[:].rearrange("p (h c2) -> p h c2", h=HPC),
                )

            def phase3(j):
                # output projection for q rows j*512..(j+1)*512
                for m in range(4):
                    mm = j * 4 + m
                    msl = slice(mm * 128, (mm + 1) * 128)
                    for n in range(2):
                        ps = aux.tile([128, 512], F32, tag="aux")
                        for cb in range(2):
                            nc.tensor.matmul(
                                ps[:],
                                out_cT[:, cb, msl],
                                wo_sb[:, cb, n * 512 : (n + 1) * 512],
                                start=(cb == 0),
                                stop=(cb == 1),
                            )
                        ob = p3s.tile([128, 512], BF16, tag="ob")
                        nc.any.tensor_copy(ob[:], ps[:])
                        nc.sync.dma_start(
                            out=outp[msl, n * 512 : (n + 1) * 512], in_=ob[:]
                        )

            # ---- phase-2 unit -------------------------------------------
            LOOKAHEAD = 2

            def unit(c, j, extras=()):
                # attention for head-pair c, q chunk j; `extras` are thunks
                # (phase-1 chains / phase-3 blocks) sprinkled between
                # i-blocks to keep the PE dense.
                jsl = slice(j * 512, (j + 1) * 512)
                out_ps = opp.tile([128, 2, 512], F32, tag="ops")
                expm_q = {}
                extras = list(extras)
                n_i = NSK + LOOKAHEAD
                stride = max(1, (n_i + len(extras)) // max(1, len(extras))) if extras else 0
                for ii in range(n_i):
                    if extras and stride and ii % stride == stride - 1:
                        extras.pop(0)()
                    if ii < NSK:
                        i = ii
                        sc = scp.tile([128, 2, 512], F32, tag="sc")
                        for h2 in range(2):
                            hsl = slice(h2 * 64, (h2 + 1) * 64)
                            nc.tensor.matmul(
                                sc[:, h2, :],
                                kT_sb[hsl, c, i * 128 : (i + 1) * 128],
                                qT_sb[hsl, c, jsl],
                                start=True,
                                stop=True,
                            )
                        expt = pexpt.tile([128, 2, 512], BF16, tag="expt")
                        nc.scalar.activation(
                            out=expt[:], in_=sc[:], func=EXP, scale=0.125
                        )
                        expm = pexpm.tile(
                            [128, 2, 512], BF16, tag="expm",
                            name=f"expm_{c}_{j}_{i}",
                        )
                        for h2 in range(2):
                            nc.vector.tensor_tensor(
                                out=expm[:, h2, :],
                                in0=expt[:, h2, :],
                                in1=mask01[:, i, jsl],
                                op=MULT,
                            )
                        expm_q[i] = expm
                    if ii >= LOOKAHEAD:
                        i = ii - LOOKAHEAD
                        expm = expm_q.pop(i)
                        for h2 in range(2):
                            h = 2 * c + h2
                            nc.tensor.matmul(
                                out_ps[:, h2, :],
                                v_aug[:, i, h * 128 : (h + 1) * 128],
                                expm[:, h2, :],
                                start=(i == 0),
                                stop=(i == NSK - 1),
                            )
                for t in extras:
                    t()
                # normalize: PSUM rows 64..127 hold Z replicated.  Copy one
                # Z row to partition 0, reciprocal there, broadcast on
                # gpsimd, multiply (HW-proven sequence; partition-shifted
                # custom-DVE ops diverge from CoreSim on hardware).
                for h2 in range(2):
                    zrow = pnorm.tile([1, 512], F32, tag="zrow")
                    nc.vector.tensor_copy(zrow[:], out_ps[64:65, h2, :])
                    zr1 = pnorm.tile([1, 512], F32, tag="zr1")
                    nc.vector.reciprocal_approx_fast(out=zr1[:], in_=zrow[:])
                    zr = pnorm.tile([64, 512], F32, tag="zr")
                    nc.gpsimd.partition_broadcast(zr[:], zr1[:])
                    nc.vector.tensor_tensor(
                        out=out_cT[h2 * 64 : (h2 + 1) * 64, c, jsl],
                        in0=out_ps[0:64, h2, :],
                        in1=zr[:],
                        op=MULT,
                    )

            # ---- emission in intended execution order -------------------
            # col-chunk groups: chains for x columns jc land right after
            # DMA chunk jc; k/q chains contract all 8 k-tiles of one
            # 512-column slice, v chains one 128-column slice.
            def K(c, jk):
                return lambda: chain_kq(wk_sb, kT_sb, c, jk)

            def Q(c, jq):
                return lambda: chain_kq(wq_sb, qT_sb, c, jq)

            groups = [
                [K(0, 0), Q(0, 0)] + [lambda s=s: chain_v(s) for s in range(0, 4)],
                [K(0, 1)] + [lambda s=s: chain_v(s) for s in range(4, 8)]
                + [K(1, 0), Q(1, 0)],
                [K(0, 2)] + [lambda s=s: chain_v(s) for s in range(8, 12)]
                + [K(1, 1), Q(0, 1)],
                [K(0, 3)] + [lambda s=s: chain_v(s) for s in range(12, 16)]
                + [K(1, 2), Q(1, 1)],
            ]
            for g in groups:
                for t in g:
                    t()

            unit(0, 0, extras=[K(1, 3), Q(0, 2), Q(1, 2)])
            unit(1, 0, extras=[Q(0, 3), Q(1, 3)])
            unit(0, 1)
            unit(1, 1)
            unit(0, 2, extras=[lambda: phase3(0)])
            unit(1, 2)
            unit(0, 3, extras=[lambda: phase3(1)])
            unit(1, 3)
            phase3(2)
            phase3(3)

    nc.compile()
    return nc


def _get_nc():
    if "nc" not in _CACHE:
        _CACHE["nc"] = _build()
    return _CACHE["nc"]


def _prep_inputs(x, mask, Wq, Wk, Wv, Wo, bo):
    """Build the 8 per-core input maps (bf16 on the wire)."""
    import ml_dtypes

    bf16 = ml_dtypes.bfloat16
    x = np.asarray(x, dtype=np.float32)
    mask = np.asarray(mask, dtype=np.int32)
    wqT = np.asarray(Wq, np.float32).T
    wkT = np.asarray(Wk, np.float32).T
    wvT = np.asarray(Wv, np.float32).T
    woT = np.asarray(Wo, np.float32).T

    xTs = [np.ascontiguousarray(x[b].T).astype(bf16) for b in range(B)]
    maskTs = [
        (np.ascontiguousarray(mask[b, 0].T) == 0).astype(bf16) for b in range(B)
    ]

    in_maps = []
    for c in range(NCORES):
        b, hq = c >> 2, c & 3
        doff = hq * DPC
        in_maps.append(
            {
                "xT": xTs[b],
                "wqT": np.ascontiguousarray(wqT[:, doff : doff + DPC]).astype(bf16),
                "wkT": np.ascontiguousarray(wkT[:, doff : doff + DPC]).astype(bf16),
                "wvT": np.ascontiguousarray(wvT[:, doff : doff + DPC]).astype(bf16),
                "woT": np.ascontiguousarray(woT[doff : doff + DPC, :]).astype(bf16),
                "maskT": maskTs[b],
            }
        )
    return in_maps


def run(inputs: dict, trace: bool = False):
    """Run the kernel; returns (full_output, BassKernelResults)."""
    from concourse.bass_utils import run_bass_kernel_spmd

    nc = _get_nc()
    in_maps = _prep_inputs(**inputs)
    res = run_bass_kernel_spmd(
        nc, in_maps, core_ids=list(range(NCORES)), trace=trace
    )
    bo = np.asarray(inputs["bo"], dtype=np.float32)
    out = np.empty((B, S, D), dtype=np.float32)
    for b in range(B):
        acc = res.results[4 * b]["outp"].astype(np.float32)
        for hq in range(1, 4):
            acc = acc + res.results[4 * b + hq]["outp"].astype(np.float32)
        out[b] = acc + bo[None, :]
    return out, res


def kernel(**inputs) -> np.ndarray:
    out, _ = run(inputs, trace=False)
    return out
